# revision 8
# baseline (speedup 1.0000x reference)
"""Trainium2 Bass kernel for the hardest-positive triplet-softplus loss.

Key observation: the reference builds the full 4096x4096 distance matrix but
only ever *uses* same-label entries (hardest-positive mining per row).  With
C=128 classes over B=4096 rows, each class has ~32 members.  Sorting rows by
label on the host makes every row's positives live in a small contiguous band
of the sorted order, so each 128-row block only needs a 224-column Gram block
instead of 4096 columns: ~18x less matmul work and ~10x less HBM traffic.

Strategy (8 NeuronCores, data-parallel over sorted row windows):
  - Host sorts rows by label (stable).  Each core owns 4 windows of 128
    consecutive sorted rows.  The 4 windows' rhs columns overlap: the core
    needs only sorted columns [cbase-56, cbase+552) = 608 columns total, so
    ONE shared fp8 x tile [128, 2pair, 608] serves all 4 windows (window w
    uses columns [128w, 128w+224), its own rows are at [128w+56, 128w+184)).
  - Per window the PE computes the [128 x 224] Gram block with 2 accumulating
    fp8-e4m3 DoubleRow matmuls (K=512 as 2 pairs of k-planes), then adds the
    host-built fp16 mask in PSUM with an identity-stationary fp16 matmul.
    mask = 256 - sq_col/2 on valid (same-label, not-self, in-range) entries,
    -30000 elsewhere; the row-max then encodes the hardest-positive distance:
        d2_ap = sq_row + 512 - 2*max_j(G[p,j] + mask[p,j])
  - DVE does one tensor_reduce(max) per window straight from PSUM.
  - Everything else is exact host numpy: d_an from the raw fp32 batch,
    softplus tail, valid mask / count, final mean.  Device output is just
    [128, 4] fp32 row-max values per core.
  - Raw bass (no TileContext): manual semaphores pinned to IDs >= 240 (the
    NEFF postamble's per-engine clear chains run after a global barrier, and
    SP's chain covers 207-255 after its own final wait), and no tile-exit
    all-engine barriers / range-clears -- the body ends on SP's single wait
    for the output-DMA semaphore, which is what gates the fixed ~7us NEFF
    postamble (Tensor's per-semaphore clear chain dominates it).
  - DMA plan (TRN2 HWDGE queues are SP and ACT only), 8 slices so the PE can
    start as soon as the first column-half of the first k-pair lands:
      SP : [xa cols 0-303] [ident+mask0] [xa cols 304-607] [mask1]
      ACT: [xb cols 0-303] [mask2]       [xb cols 304-607] [mask3]
"""

import os
import sys

import numpy as np

for _p in ("/opt/trn_rl_repo", "/root/.axon_site/_ro/trn_rl_repo"):
    if os.path.isdir(_p) and _p not in sys.path:
        sys.path.append(_p)

import ml_dtypes  # noqa: E402

import concourse.bass as bass  # noqa: E402
import concourse.bacc as bacc  # noqa: E402
from concourse import mybir  # noqa: E402
from concourse import bass_utils  # noqa: E402

B = 4096
DIM = 512
C = 128
TEMP = 0.05
NCORES = 8
NW = 4            # windows of 128 sorted rows per core
W = 224           # rhs columns per window (own 128 rows + 56/40 pad)
NK = DIM // 128   # 4 contraction k-planes of 128
LHS0 = 56         # offset of a window's own rows inside its W columns
NEG = -30000.0    # mask value for non-positive columns
XCOLS = 128 * (NW - 1) + W            # 608 shared x columns per core
XH = XCOLS // 2                       # 304: column half
XPB = 2 * XCOLS                       # 1216 bytes/partition per k-plane pair
MB = 2 * W                            # 448 bytes/partition per window mask
IDB = 256                             # identity: 128 f16 per partition
TOT = 2 * XPB + IDB + NW * MB         # 4480 bytes/partition total

# DRAM per-partition layout (offset, nbytes):
D_XA1 = (0, 2 * XH)                   # xa plane0[0:304] | plane1[0:304]
D_IM0 = (608, IDB + MB)               # identity | mask0
D_XA2 = (1312, 2 * XH)
D_M1 = (1920, MB)
D_XB1 = (2368, 2 * XH)
D_M2 = (2976, MB)
D_XB2 = (3424, 2 * XH)
D_M3 = (4032, MB)

# SBUF per-partition layout inside `buf`:
S_XA = 0                              # [2 x 608] fp8 pair 0
S_XB = XPB                            # [2 x 608] fp8 pair 1
S_ID = 2 * XPB                        # [128] f16 identity
S_M = 2 * XPB + IDB                   # 4 x [224] f16 masks

F32 = mybir.dt.float32
F16 = mybir.dt.float16
E4M3 = mybir.dt.float8e4
U8 = mybir.dt.uint8
ALU = mybir.AluOpType

_NC_CACHE = None


def _build_nc():
    nc = bacc.Bacc(
        "TRN2",
        target_bir_lowering=False,
        debug=False,
        enable_asserts=False,
    )

    cw_d = nc.dram_tensor("cw", [128, TOT], U8, kind="ExternalInput").ap()
    out_d = nc.dram_tensor("out", [128, NW], F32, kind="ExternalOutput").ap()

    # All sems >= 240: cleared only in the postamble, after a global barrier.
    sd = [nc.alloc_semaphore(f"sd{i}", num=240 + i) for i in range(8)]
    s_xa1, s_im0, s_xa2, s_m1, s_xb1, s_m2, s_xb2, s_m3 = sd
    s_pe = nc.alloc_semaphore("s_pe", num=248)
    s_dv = nc.alloc_semaphore("s_dv", num=249)
    s_out = nc.alloc_semaphore("s_out", num=250)

    buf = nc.alloc_sbuf_tensor("buf", [128, TOT], U8)
    outt = nc.alloc_sbuf_tensor("outt", [128, NW], F32)
    accs = [nc.alloc_psum_tensor(f"acc{w}", [128, W], F32) for w in range(NW)]

    bp = buf.ap()
    xa_u8 = bp[:, S_XA:S_XA + XPB].rearrange("p (two f) -> p two f", two=2)
    xb_u8 = bp[:, S_XB:S_XB + XPB].rearrange("p (two f) -> p two f", two=2)

    def dma(engine, sem, dst_ap, src):
        off, nb = src
        engine.dma_start(dst_ap, cw_d[:, off:off + nb]).then_inc(sem, 16)

    # SP queue: xa first half, ident+mask0, xa second half, mask1
    dma(nc.sync, s_xa1, xa_u8[:, :, 0:XH], D_XA1)
    dma(nc.sync, s_im0, bp[:, S_ID:S_ID + IDB + MB], D_IM0)
    dma(nc.sync, s_xa2, xa_u8[:, :, XH:XCOLS], D_XA2)
    dma(nc.sync, s_m1, bp[:, S_M + MB:S_M + 2 * MB], D_M1)
    # ACT queue: xb first half, mask2, xb second half, mask3
    dma(nc.scalar, s_xb1, xb_u8[:, :, 0:XH], D_XB1)
    dma(nc.scalar, s_m2, bp[:, S_M + 2 * MB:S_M + 3 * MB], D_M2)
    dma(nc.scalar, s_xb2, xb_u8[:, :, XH:XCOLS], D_XB2)
    dma(nc.scalar, s_m3, bp[:, S_M + 3 * MB:S_M + 4 * MB], D_M3)

    xa3 = bp[:, S_XA:S_XA + XPB].bitcast(E4M3).rearrange(
        "p (two f) -> p two f", two=2)
    xb3 = bp[:, S_XB:S_XB + XPB].bitcast(E4M3).rearrange(
        "p (two f) -> p two f", two=2)
    identap = bp[:, S_ID:S_ID + IDB].bitcast(F16)

    DR = mybir.MatmulPerfMode.DoubleRow
    mask_sems = [s_im0, s_m1, s_m2, s_m3]
    for w in range(NW):
        c0 = 128 * w
        mm0 = nc.tensor.matmul(
            accs[w].ap(),
            xa3[:, :, c0 + LHS0:c0 + LHS0 + 128],
            xa3[:, :, c0:c0 + W],
            start=True, stop=False, perf_mode=DR,
        )
        if w == 0:
            mm0._wait_ge(s_xa1, 16)
        elif w == 1:
            mm0._wait_ge(s_xa2, 16)
        mm1 = nc.tensor.matmul(
            accs[w].ap(),
            xb3[:, :, c0 + LHS0:c0 + LHS0 + 128],
            xb3[:, :, c0:c0 + W],
            start=False, stop=False, perf_mode=DR,
        )
        if w == 0:
            mm1._wait_ge(s_xb1, 16)
        elif w == 1:
            mm1._wait_ge(s_xb2, 16)
        # PSUM += I @ mask_w  (adds the row-dependent mask on the PE)
        mk = bp[:, S_M + MB * w:S_M + MB * (w + 1)].bitcast(F16)
        mmi = nc.tensor.matmul(
            accs[w].ap(), identap, mk, start=False, stop=True)
        mmi._wait_ge(mask_sems[w], 16)
        mmi.then_inc(s_pe, 1)

    for w in range(NW):
        tr = nc.vector.tensor_reduce(
            outt.ap()[:, w:w + 1], accs[w].ap(),
            axis=mybir.AxisListType.X, op=ALU.max)
        tr._wait_ge(s_pe, w + 1)
        tr.then_inc(s_dv, 1)

    od = nc.sync.dma_start(out_d, outt.ap())
    od._wait_ge(s_dv, NW)
    od.then_inc(s_out, 16)
    nc.sync.wait_ge(s_out, 16)

    nc.compile()
    return nc


def get_nc():
    global _NC_CACHE
    if _NC_CACHE is None:
        _NC_CACHE = _build_nc()
    return _NC_CACHE


def _prep_inputs(batch, labels, anchors=None, negatives=None):
    """Host-side prep: per-core window tensors + (order, sqs) for unshard."""
    batch = np.ascontiguousarray(np.asarray(batch), dtype=np.float32)
    labels = np.asarray(labels).astype(np.int64)

    order = np.argsort(labels, kind="stable").astype(np.int64)
    slab = labels[order]
    xs = batch[order]
    sqs = np.einsum("ij,ij->i", xs, xs, dtype=np.float64)

    xsT = np.ascontiguousarray(xs.T.astype(ml_dtypes.float8_e4m3))   # [DIM, B]
    maskvals = 256.0 - sqs / 2.0                                      # [B] f64
    ident_bytes = np.eye(128, dtype=np.float16).view(np.uint8)        # [128,256]

    # containment: every row's class fits in its window's W columns
    starts = np.searchsorted(slab, slab, side="left")
    ends = np.searchsorted(slab, slab, side="right")

    in_maps = []
    for c in range(NCORES):
        cw = np.empty((128, TOT), np.uint8)
        colbase = c * 512 - LHS0
        colpos = colbase + np.arange(XCOLS)
        validc = (colpos >= 0) & (colpos < B)
        cp = np.clip(colpos, 0, B - 1)
        # x planes: xplane[t] = fp8 bytes of contraction dims t*128+p
        xplane = xsT[:, cp].reshape(NK, 128, XCOLS).view(np.uint8)
        for (off, nb), pair, sl in (
            (D_XA1, 0, slice(0, XH)),
            (D_XA2, 0, slice(XH, XCOLS)),
            (D_XB1, 1, slice(0, XH)),
            (D_XB2, 1, slice(XH, XCOLS)),
        ):
            half = np.concatenate(
                [xplane[2 * pair, :, sl], xplane[2 * pair + 1, :, sl]], axis=1)
            cw[:, off:off + nb] = half
        masks = []
        for wl in range(NW):
            base = (c * NW + wl) * 128
            assert starts[base] >= base - LHS0, "class overflows window left pad"
            assert ends[base + 127] <= base + (W - LHS0), (
                "class overflows window right pad")
            wcol = colpos[128 * wl:128 * wl + W]
            wvalid = validc[128 * wl:128 * wl + W]
            wcp = cp[128 * wl:128 * wl + W]
            rowpos = base + np.arange(128)
            ok = (wvalid[None, :]
                  & (slab[wcp][None, :] == slab[rowpos][:, None])
                  & (wcol[None, :] != rowpos[:, None]))
            mask = np.where(ok, maskvals[wcp][None, :], NEG).astype(np.float16)
            masks.append(mask.view(np.uint8).reshape(128, MB))
        cw[:, D_IM0[0]:D_IM0[0] + IDB] = ident_bytes
        cw[:, D_IM0[0] + IDB:D_IM0[0] + IDB + MB] = masks[0]
        cw[:, D_M1[0]:D_M1[0] + MB] = masks[1]
        cw[:, D_M2[0]:D_M2[0] + MB] = masks[2]
        cw[:, D_M3[0]:D_M3[0] + MB] = masks[3]
        in_maps.append({"cw": cw})
    return in_maps, order, sqs


def kernel(batch, labels, anchors=None, negatives=None, **_kwargs):
    batch = np.ascontiguousarray(np.asarray(batch), dtype=np.float32)
    labels_np = np.asarray(labels).astype(np.int64)
    negatives_np = np.asarray(negatives).astype(np.int64)

    in_maps, order, sqs = _prep_inputs(batch, labels_np)
    nc = get_nc()
    res = bass_utils.run_bass_kernel_spmd(nc, in_maps, core_ids=list(range(NCORES)))

    v = np.stack([np.asarray(r["out"], dtype=np.float64) for r in res.results])
    vsorted = v.transpose(0, 2, 1).reshape(B)     # [core, w, p] -> sorted pos
    d2ap_sorted = sqs + 512.0 - 2.0 * vsorted
    d2_ap = np.empty(B, dtype=np.float64)
    d2_ap[order] = d2ap_sorted
    d_ap = np.sqrt(np.maximum(d2_ap, 1e-12))

    diff = batch.astype(np.float64) - batch[negatives_np].astype(np.float64)
    d_an = np.sqrt(np.maximum(np.einsum("ij,ij->i", diff, diff), 1e-12))

    z = (d_ap - d_an) / (2.0 * TEMP)
    per = np.logaddexp(0.0, z)

    hist = np.bincount(labels_np, minlength=C)
    valid = (hist[labels_np] - 1) > 1
    count = float(valid.sum())
    loss = float((per * valid.astype(np.float64)).sum() / count)
    return np.array([loss], dtype=np.float32)


# revision 10
# speedup vs baseline: 1.0343x; 1.0343x over previous
"""Trainium2 Bass kernel for the hardest-positive triplet-softplus loss.

Key observation: the reference builds the full 4096x4096 distance matrix but
only ever *uses* same-label entries (hardest-positive mining per row).  With
C=128 classes over B=4096 rows, each class has ~32 members.  Sorting rows by
label on the host makes every row's positives live in a small contiguous band
of the sorted order, so each 128-row block only needs a 224-column Gram block
instead of 4096 columns: ~18x less matmul work and ~10x less HBM traffic.

Strategy (8 NeuronCores, data-parallel over sorted row windows):
  - Host sorts rows by label (stable).  Each core owns 4 windows of 128
    consecutive sorted rows.  The 4 windows' rhs columns overlap: the core
    needs only sorted columns [cbase-56, cbase+552) = 608 columns total, so
    ONE shared fp8 x tile [128, 2pair, 608] serves all 4 windows (window w
    uses columns [128w, 128w+224), its own rows are at [128w+56, 128w+184)).
  - Per window the PE computes the [128 x 224] Gram block with 2 accumulating
    fp8-e4m3 DoubleRow matmuls (K=512 as 2 pairs of k-planes), then adds the
    host-built fp16 mask in PSUM with an identity-stationary fp16 matmul.
    mask = 256 - sq_col/2 on valid (same-label, not-self, in-range) entries,
    -30000 elsewhere; the row-max then encodes the hardest-positive distance:
        d2_ap = sq_row + 512 - 2*max_j(G[p,j] + mask[p,j])
  - DVE does one tensor_reduce(max) per window straight from PSUM.
  - Everything else is exact host numpy: d_an from the raw fp32 batch,
    softplus tail, valid mask / count, final mean.  Device output is just
    [128, 4] fp32 row-max values per core.
  - Raw bass (no TileContext): manual semaphores pinned to IDs >= 240 (the
    NEFF postamble's per-engine clear chains run after a global barrier, and
    SP's chain covers 207-255 after its own final wait), and no tile-exit
    all-engine barriers / range-clears -- the body ends on SP's single wait
    for the output-DMA semaphore, which is what gates the fixed ~7us NEFF
    postamble (Tensor's per-semaphore clear chain dominates it).
  - DMA plan (TRN2 HWDGE queues are SP and ACT only), 3 slices per queue so
    the PE can start as soon as window 0's first k-pair columns land:
      SP : [xa cols 0-223] [xa cols 224-607] [mask2+mask3]
      ACT: [xb cols 0-223] [ident+mask0+mask1] [xb cols 224-607]
    The output DMA carries no completion semaphore: the NEFF postamble (a
    global barrier plus ~6us of per-semaphore clears) runs after its issue
    and orders NEFF completion far behind the 2KB transfer.
"""

import os
import sys

import numpy as np

for _p in ("/opt/trn_rl_repo", "/root/.axon_site/_ro/trn_rl_repo"):
    if os.path.isdir(_p) and _p not in sys.path:
        sys.path.append(_p)

import ml_dtypes  # noqa: E402

import concourse.bass as bass  # noqa: E402
import concourse.bacc as bacc  # noqa: E402
from concourse import mybir  # noqa: E402
from concourse import bass_utils  # noqa: E402

B = 4096
DIM = 512
C = 128
TEMP = 0.05
NCORES = 8
NW = 4            # windows of 128 sorted rows per core
W = 224           # rhs columns per window (own 128 rows + 56/40 pad)
NK = DIM // 128   # 4 contraction k-planes of 128
LHS0 = 56         # offset of a window's own rows inside its W columns
NEG = -30000.0    # mask value for non-positive columns
XCOLS = 128 * (NW - 1) + W            # 608 shared x columns per core
XH = W                                # 224: first column slice (= window 0)
XPB = 2 * XCOLS                       # 1216 bytes/partition per k-plane pair
MB = 2 * W                            # 448 bytes/partition per window mask
IDB = 256                             # identity: 128 f16 per partition
TOT = 2 * XPB + IDB + NW * MB         # 4480 bytes/partition total

# DRAM per-partition layout (offset, nbytes):
D_XA1 = (0, 2 * XH)                   # xa plane0[0:224] | plane1[0:224]
D_XA2 = (448, 2 * (XCOLS - XH))       # xa plane0[224:608] | plane1[224:608]
D_M23 = (1216, 2 * MB)                # mask2 | mask3
D_XB1 = (2112, 2 * XH)                # xb first slice
D_IM01 = (2560, IDB + 2 * MB)         # identity | mask0 | mask1
D_XB2 = (3712, 2 * (XCOLS - XH))      # xb second slice

# SBUF per-partition layout inside `buf`:
S_XA = 0                              # [2 x 608] fp8 pair 0
S_XB = XPB                            # [2 x 608] fp8 pair 1
S_ID = 2 * XPB                        # [128] f16 identity
S_M = 2 * XPB + IDB                   # 4 x [224] f16 masks

F32 = mybir.dt.float32
F16 = mybir.dt.float16
E4M3 = mybir.dt.float8e4
U8 = mybir.dt.uint8
ALU = mybir.AluOpType

_NC_CACHE = None


def _build_nc():
    nc = bacc.Bacc(
        "TRN2",
        target_bir_lowering=False,
        debug=False,
        enable_asserts=False,
    )

    cw_d = nc.dram_tensor("cw", [128, TOT], U8, kind="ExternalInput").ap()
    out_d = nc.dram_tensor("out", [128, NW], F32, kind="ExternalOutput").ap()

    # All sems >= 240: cleared only in the postamble, after a global barrier.
    sd = [nc.alloc_semaphore(f"sd{i}", num=240 + i) for i in range(6)]
    s_xa1, s_xa2, s_m23, s_xb1, s_im01, s_xb2 = sd
    s_pe = nc.alloc_semaphore("s_pe", num=248)
    s_dv = nc.alloc_semaphore("s_dv", num=249)
    s_out = nc.alloc_semaphore("s_out", num=250)

    buf = nc.alloc_sbuf_tensor("buf", [128, TOT], U8)
    outt = nc.alloc_sbuf_tensor("outt", [128, NW], F32)
    accs = [nc.alloc_psum_tensor(f"acc{w}", [128, W], F32) for w in range(NW)]

    bp = buf.ap()
    xa_u8 = bp[:, S_XA:S_XA + XPB].rearrange("p (two f) -> p two f", two=2)
    xb_u8 = bp[:, S_XB:S_XB + XPB].rearrange("p (two f) -> p two f", two=2)

    def dma(engine, sem, dst_ap, src):
        off, nb = src
        engine.dma_start(dst_ap, cw_d[:, off:off + nb]).then_inc(sem, 16)

    # SP queue: xa first slice, xa rest, masks 2+3
    dma(nc.sync, s_xa1, xa_u8[:, :, 0:XH], D_XA1)
    dma(nc.sync, s_xa2, xa_u8[:, :, XH:XCOLS], D_XA2)
    dma(nc.sync, s_m23, bp[:, S_M + 2 * MB:S_M + 4 * MB], D_M23)
    # ACT queue: xb first slice, ident+masks 0+1, xb rest
    dma(nc.scalar, s_xb1, xb_u8[:, :, 0:XH], D_XB1)
    dma(nc.scalar, s_im01, bp[:, S_ID:S_ID + IDB + 2 * MB], D_IM01)
    dma(nc.scalar, s_xb2, xb_u8[:, :, XH:XCOLS], D_XB2)

    xa3 = bp[:, S_XA:S_XA + XPB].bitcast(E4M3).rearrange(
        "p (two f) -> p two f", two=2)
    xb3 = bp[:, S_XB:S_XB + XPB].bitcast(E4M3).rearrange(
        "p (two f) -> p two f", two=2)
    identap = bp[:, S_ID:S_ID + IDB].bitcast(F16)

    DR = mybir.MatmulPerfMode.DoubleRow
    mask_sems = [s_im01, None, s_m23, None]
    for w in range(NW):
        c0 = 128 * w
        mm0 = nc.tensor.matmul(
            accs[w].ap(),
            xa3[:, :, c0 + LHS0:c0 + LHS0 + 128],
            xa3[:, :, c0:c0 + W],
            start=True, stop=False, perf_mode=DR,
        )
        if w == 0:
            mm0._wait_ge(s_xa1, 16)
        elif w == 1:
            mm0._wait_ge(s_xa2, 16)
        mm1 = nc.tensor.matmul(
            accs[w].ap(),
            xb3[:, :, c0 + LHS0:c0 + LHS0 + 128],
            xb3[:, :, c0:c0 + W],
            start=False, stop=False, perf_mode=DR,
        )
        if w == 0:
            mm1._wait_ge(s_xb1, 16)
        elif w == 1:
            mm1._wait_ge(s_xb2, 16)
        # PSUM += I @ mask_w  (adds the row-dependent mask on the PE)
        mk = bp[:, S_M + MB * w:S_M + MB * (w + 1)].bitcast(F16)
        mmi = nc.tensor.matmul(
            accs[w].ap(), identap, mk, start=False, stop=True)
        if mask_sems[w] is not None:
            mmi._wait_ge(mask_sems[w], 16)
        mmi.then_inc(s_pe, 1)

    for w in range(NW):
        tr = nc.vector.tensor_reduce(
            outt.ap()[:, w:w + 1], accs[w].ap(),
            axis=mybir.AxisListType.X, op=ALU.max)
        tr._wait_ge(s_pe, w + 1)
        tr.then_inc(s_dv, 1)

    od = nc.sync.dma_start(out_d, outt.ap())
    od._wait_ge(s_dv, NW)
    od.then_inc(s_out, 16)   # unwaited; postamble clears it

    nc.compile()
    return nc


def get_nc():
    global _NC_CACHE
    if _NC_CACHE is None:
        _NC_CACHE = _build_nc()
    return _NC_CACHE


def _prep_inputs(batch, labels, anchors=None, negatives=None):
    """Host-side prep: per-core window tensors + (order, sqs) for unshard."""
    batch = np.ascontiguousarray(np.asarray(batch), dtype=np.float32)
    labels = np.asarray(labels).astype(np.int64)

    order = np.argsort(labels, kind="stable").astype(np.int64)
    slab = labels[order]
    xs = batch[order]
    sqs = np.einsum("ij,ij->i", xs, xs, dtype=np.float64)

    xsT = np.ascontiguousarray(xs.T.astype(ml_dtypes.float8_e4m3))   # [DIM, B]
    maskvals = 256.0 - sqs / 2.0                                      # [B] f64
    ident_bytes = np.eye(128, dtype=np.float16).view(np.uint8)        # [128,256]

    # containment: every row's class fits in its window's W columns
    starts = np.searchsorted(slab, slab, side="left")
    ends = np.searchsorted(slab, slab, side="right")

    in_maps = []
    for c in range(NCORES):
        cw = np.empty((128, TOT), np.uint8)
        colbase = c * 512 - LHS0
        colpos = colbase + np.arange(XCOLS)
        validc = (colpos >= 0) & (colpos < B)
        cp = np.clip(colpos, 0, B - 1)
        # x planes: xplane[t] = fp8 bytes of contraction dims t*128+p
        xplane = xsT[:, cp].reshape(NK, 128, XCOLS).view(np.uint8)
        for (off, nb), pair, sl in (
            (D_XA1, 0, slice(0, XH)),
            (D_XA2, 0, slice(XH, XCOLS)),
            (D_XB1, 1, slice(0, XH)),
            (D_XB2, 1, slice(XH, XCOLS)),
        ):
            part = np.concatenate(
                [xplane[2 * pair, :, sl], xplane[2 * pair + 1, :, sl]], axis=1)
            cw[:, off:off + nb] = part
        masks = []
        for wl in range(NW):
            base = (c * NW + wl) * 128
            assert starts[base] >= base - LHS0, "class overflows window left pad"
            assert ends[base + 127] <= base + (W - LHS0), (
                "class overflows window right pad")
            wcol = colpos[128 * wl:128 * wl + W]
            wvalid = validc[128 * wl:128 * wl + W]
            wcp = cp[128 * wl:128 * wl + W]
            rowpos = base + np.arange(128)
            ok = (wvalid[None, :]
                  & (slab[wcp][None, :] == slab[rowpos][:, None])
                  & (wcol[None, :] != rowpos[:, None]))
            mask = np.where(ok, maskvals[wcp][None, :], NEG).astype(np.float16)
            masks.append(mask.view(np.uint8).reshape(128, MB))
        o = D_IM01[0]
        cw[:, o:o + IDB] = ident_bytes
        cw[:, o + IDB:o + IDB + MB] = masks[0]
        cw[:, o + IDB + MB:o + IDB + 2 * MB] = masks[1]
        cw[:, D_M23[0]:D_M23[0] + MB] = masks[2]
        cw[:, D_M23[0] + MB:D_M23[0] + 2 * MB] = masks[3]
        in_maps.append({"cw": cw})
    return in_maps, order, sqs


def kernel(batch, labels, anchors=None, negatives=None, **_kwargs):
    batch = np.ascontiguousarray(np.asarray(batch), dtype=np.float32)
    labels_np = np.asarray(labels).astype(np.int64)
    negatives_np = np.asarray(negatives).astype(np.int64)

    in_maps, order, sqs = _prep_inputs(batch, labels_np)
    nc = get_nc()
    res = bass_utils.run_bass_kernel_spmd(nc, in_maps, core_ids=list(range(NCORES)))

    v = np.stack([np.asarray(r["out"], dtype=np.float64) for r in res.results])
    vsorted = v.transpose(0, 2, 1).reshape(B)     # [core, w, p] -> sorted pos
    d2ap_sorted = sqs + 512.0 - 2.0 * vsorted
    d2_ap = np.empty(B, dtype=np.float64)
    d2_ap[order] = d2ap_sorted
    d_ap = np.sqrt(np.maximum(d2_ap, 1e-12))

    diff = batch.astype(np.float64) - batch[negatives_np].astype(np.float64)
    d_an = np.sqrt(np.maximum(np.einsum("ij,ij->i", diff, diff), 1e-12))

    z = (d_ap - d_an) / (2.0 * TEMP)
    per = np.logaddexp(0.0, z)

    hist = np.bincount(labels_np, minlength=C)
    valid = (hist[labels_np] - 1) > 1
    count = float(valid.sum())
    loss = float((per * valid.astype(np.float64)).sum() / count)
    return np.array([loss], dtype=np.float32)


# revision 13
# speedup vs baseline: 1.0708x; 1.0353x over previous
"""Trainium2 Bass kernel for the hardest-positive triplet-softplus loss.

Key observation: the reference builds the full 4096x4096 distance matrix but
only ever *uses* same-label entries (hardest-positive mining per row).  With
C=128 classes over B=4096 rows, each class has ~32 members.  Sorting rows by
label on the host makes every row's positives live in a small contiguous band
of the sorted order, so each 128-row block only needs a 224-column Gram block
instead of 4096 columns: ~18x less matmul work and ~10x less HBM traffic.

Strategy (8 NeuronCores, data-parallel over sorted row windows):
  - Host sorts rows by label (stable).  Each core owns 4 windows of 128
    consecutive sorted rows.  The 4 windows' rhs columns overlap: the core
    needs only sorted columns [cbase-56, cbase+552) = 608 columns total, so
    ONE shared fp8 x tile [128, 2pair, 608] serves all 4 windows (window w
    uses columns [128w, 128w+224), its own rows are at [128w+56, 128w+184)).
  - Per window the PE computes the [128 x 224] Gram block with 2 accumulating
    fp8-e4m3 DoubleRow matmuls (K=512 as 2 pairs of k-planes), then adds the
    host-built fp16 mask in PSUM with an identity-stationary fp16 matmul.
    mask = 256 - sq_col/2 on valid (same-label, not-self, in-range) entries,
    -30000 elsewhere; the row-max then encodes the hardest-positive distance:
        d2_ap = sq_row + 512 - 2*max_j(G[p,j] + mask[p,j])
  - DVE does one tensor_reduce(max) per window straight from PSUM.
  - Everything else is exact host numpy: d_an from the raw fp32 batch,
    softplus tail, valid mask / count, final mean.  Device output is just
    [128, 4] fp32 row-max values per core.
  - Raw bass (no TileContext): manual semaphores pinned to IDs >= 240 (the
    NEFF postamble's per-engine clear chains run after a global barrier, and
    SP's chain covers 207-255 after its own final wait), and no tile-exit
    all-engine barriers / range-clears -- the body ends on SP's single wait
    for the output-DMA semaphore, which is what gates the fixed ~7us NEFF
    postamble (Tensor's per-semaphore clear chain dominates it).
  - DMA plan (TRN2 HWDGE queues are SP and ACT only); each x pair is two
    single-run slices split at byte 832 so the first slice covers exactly
    what window 0 needs and the PE starts as early as possible:
      SP : [xa bytes 0:832] [ident+mask0] [xa bytes 832:1216] [mask1]
      ACT: [xb bytes 0:832] [xb bytes 832:1216] [mask2+mask3]
    The output DMA carries no completion semaphore: the NEFF postamble (a
    global barrier plus ~6us of per-semaphore clears) runs after its issue
    and orders NEFF completion far behind the 2KB transfer.
"""

import os
import sys

import numpy as np

for _p in ("/opt/trn_rl_repo", "/root/.axon_site/_ro/trn_rl_repo"):
    if os.path.isdir(_p) and _p not in sys.path:
        sys.path.append(_p)

import ml_dtypes  # noqa: E402

import concourse.bass as bass  # noqa: E402
import concourse.bacc as bacc  # noqa: E402
from concourse import mybir  # noqa: E402
from concourse import bass_utils  # noqa: E402

B = 4096
DIM = 512
C = 128
TEMP = 0.05
NCORES = 8
NW = 4            # windows of 128 sorted rows per core
W = 224           # rhs columns per window (own 128 rows + 56/40 pad)
NK = DIM // 128   # 4 contraction k-planes of 128
LHS0 = 56         # offset of a window's own rows inside its W columns
NEG = -30000.0    # mask value for non-positive columns
XCOLS = 128 * (NW - 1) + W            # 608 shared x columns per core
XH = W                                # 224: first column slice (= window 0)
XPB = 2 * XCOLS                       # 1216 bytes/partition per k-plane pair
MB = 2 * W                            # 448 bytes/partition per window mask
IDB = 256                             # identity: 128 f16 per partition
TOT = 2 * XPB + IDB + NW * MB         # 4480 bytes/partition total

# DRAM per-partition layout (offset, nbytes).  The x pairs keep the
# plane-blocked SBUF layout [plane_even 608 | plane_odd 608] the DR matmul
# needs, but each pair is DMA'd as two single-run slices split at byte 832:
# the first slice [0:832) = plane_even full + plane_odd cols [0:224) covers
# everything window 0 needs, so the PE starts early, and every slice is one
# contiguous run per partition (big DMA packets).
XSPLIT = XCOLS + XH                   # 832: first-slice bytes of a pair
D_XA1 = (0, XSPLIT)                   # xa bytes [0:832)
D_IDM0 = (832, IDB + MB)              # identity | mask0
D_XA2 = (1536, XPB - XSPLIT)          # xa bytes [832:1216)
D_M1 = (1920, MB)                     # mask1
D_XB1 = (2368, XSPLIT)                # xb bytes [0:832)
D_XB2 = (3200, XPB - XSPLIT)          # xb bytes [832:1216)
D_M23 = (3584, 2 * MB)                # mask2 | mask3

# SBUF per-partition layout inside `buf`:
S_XA = 0                              # [2 x 608] fp8 pair 0
S_XB = XPB                            # [2 x 608] fp8 pair 1
S_ID = 2 * XPB                        # [128] f16 identity
S_M = 2 * XPB + IDB                   # 4 x [224] f16 masks

F32 = mybir.dt.float32
F16 = mybir.dt.float16
E4M3 = mybir.dt.float8e4
U8 = mybir.dt.uint8
ALU = mybir.AluOpType

_NC_CACHE = None


def _build_nc():
    nc = bacc.Bacc(
        "TRN2",
        target_bir_lowering=False,
        debug=False,
        enable_asserts=False,
    )

    cw_d = nc.dram_tensor("cw", [128, TOT], U8, kind="ExternalInput").ap()
    out_d = nc.dram_tensor("out", [128, NW], F32, kind="ExternalOutput").ap()

    # All sems >= 240: cleared only in the postamble, after a global barrier.
    sd = [nc.alloc_semaphore(f"sd{i}", num=240 + i) for i in range(7)]
    s_xa1, s_idm0, s_xa2, s_m1, s_xb1, s_xb2, s_m23 = sd
    s_pe = nc.alloc_semaphore("s_pe", num=248)
    s_dv = nc.alloc_semaphore("s_dv", num=249)
    s_out = nc.alloc_semaphore("s_out", num=250)

    buf = nc.alloc_sbuf_tensor("buf", [128, TOT], U8)
    outt = nc.alloc_sbuf_tensor("outt", [128, NW], F32)
    accs = [nc.alloc_psum_tensor(f"acc{w}", [128, W], F32) for w in range(NW)]

    bp = buf.ap()

    def dma(engine, sem, sbuf_off, src):
        off, nb = src
        engine.dma_start(
            bp[:, sbuf_off:sbuf_off + nb], cw_d[:, off:off + nb]
        ).then_inc(sem, 16)

    # SP queue: xa window-0 slice, ident+mask0, xa rest, mask1
    dma(nc.sync, s_xa1, S_XA, D_XA1)
    dma(nc.sync, s_idm0, S_ID, D_IDM0)
    dma(nc.sync, s_xa2, S_XA + XSPLIT, D_XA2)
    dma(nc.sync, s_m1, S_M + MB, D_M1)
    # ACT queue: xb window-0 slice, xb rest, masks 2+3
    dma(nc.scalar, s_xb1, S_XB, D_XB1)
    dma(nc.scalar, s_xb2, S_XB + XSPLIT, D_XB2)
    dma(nc.scalar, s_m23, S_M + 2 * MB, D_M23)

    xa3 = bp[:, S_XA:S_XA + XPB].bitcast(E4M3).rearrange(
        "p (two f) -> p two f", two=2)
    xb3 = bp[:, S_XB:S_XB + XPB].bitcast(E4M3).rearrange(
        "p (two f) -> p two f", two=2)
    identap = bp[:, S_ID:S_ID + IDB].bitcast(F16)

    DR = mybir.MatmulPerfMode.DoubleRow
    mask_sems = [s_idm0, s_m1, s_m23, None]
    # noqa: waits below reference sems in DMA-queue order
    for w in range(NW):
        c0 = 128 * w
        mm0 = nc.tensor.matmul(
            accs[w].ap(),
            xa3[:, :, c0 + LHS0:c0 + LHS0 + 128],
            xa3[:, :, c0:c0 + W],
            start=True, stop=False, perf_mode=DR,
        )
        if w == 0:
            mm0._wait_ge(s_xa1, 16)
        elif w == 1:
            mm0._wait_ge(s_xa2, 16)
        mm1 = nc.tensor.matmul(
            accs[w].ap(),
            xb3[:, :, c0 + LHS0:c0 + LHS0 + 128],
            xb3[:, :, c0:c0 + W],
            start=False, stop=False, perf_mode=DR,
        )
        if w == 0:
            mm1._wait_ge(s_xb1, 16)
        elif w == 1:
            mm1._wait_ge(s_xb2, 16)
        # PSUM += I @ mask_w  (adds the row-dependent mask on the PE)
        mk = bp[:, S_M + MB * w:S_M + MB * (w + 1)].bitcast(F16)
        mmi = nc.tensor.matmul(
            accs[w].ap(), identap, mk, start=False, stop=True)
        if mask_sems[w] is not None:
            mmi._wait_ge(mask_sems[w], 16)
        mmi.then_inc(s_pe, 1)

    for w in range(NW):
        tr = nc.vector.tensor_reduce(
            outt.ap()[:, w:w + 1], accs[w].ap(),
            axis=mybir.AxisListType.X, op=ALU.max)
        tr._wait_ge(s_pe, w + 1)
        tr.then_inc(s_dv, 1)

    od = nc.sync.dma_start(out_d, outt.ap())
    od._wait_ge(s_dv, NW)
    od.then_inc(s_out, 16)   # unwaited; postamble clears it

    nc.compile()
    return nc


def get_nc():
    global _NC_CACHE
    if _NC_CACHE is None:
        _NC_CACHE = _build_nc()
    return _NC_CACHE


def _prep_inputs(batch, labels, anchors=None, negatives=None):
    """Host-side prep: per-core window tensors + (order, sqs) for unshard."""
    batch = np.ascontiguousarray(np.asarray(batch), dtype=np.float32)
    labels = np.asarray(labels).astype(np.int64)

    order = np.argsort(labels, kind="stable").astype(np.int64)
    slab = labels[order]
    xs = batch[order]
    sqs = np.einsum("ij,ij->i", xs, xs, dtype=np.float64)

    xsT = np.ascontiguousarray(xs.T.astype(ml_dtypes.float8_e4m3))   # [DIM, B]
    maskvals = 256.0 - sqs / 2.0                                      # [B] f64
    ident_bytes = np.eye(128, dtype=np.float16).view(np.uint8)        # [128,256]

    # containment: every row's class fits in its window's W columns
    starts = np.searchsorted(slab, slab, side="left")
    ends = np.searchsorted(slab, slab, side="right")

    in_maps = []
    for c in range(NCORES):
        cw = np.empty((128, TOT), np.uint8)
        colbase = c * 512 - LHS0
        colpos = colbase + np.arange(XCOLS)
        validc = (colpos >= 0) & (colpos < B)
        cp = np.clip(colpos, 0, B - 1)
        # x planes: xplane[t] = fp8 bytes of contraction dims t*128+p
        xplane = xsT[:, cp].reshape(NK, 128, XCOLS).view(np.uint8)
        for (off1, off2), pair in ((D_XA1[0], D_XA2[0]), 0), (
                (D_XB1[0], D_XB2[0]), 1):
            part = np.concatenate(
                [xplane[2 * pair], xplane[2 * pair + 1]], axis=1)  # [128,1216]
            cw[:, off1:off1 + XSPLIT] = part[:, :XSPLIT]
            cw[:, off2:off2 + XPB - XSPLIT] = part[:, XSPLIT:]
        masks = []
        for wl in range(NW):
            base = (c * NW + wl) * 128
            assert starts[base] >= base - LHS0, "class overflows window left pad"
            assert ends[base + 127] <= base + (W - LHS0), (
                "class overflows window right pad")
            wcol = colpos[128 * wl:128 * wl + W]
            wvalid = validc[128 * wl:128 * wl + W]
            wcp = cp[128 * wl:128 * wl + W]
            rowpos = base + np.arange(128)
            ok = (wvalid[None, :]
                  & (slab[wcp][None, :] == slab[rowpos][:, None])
                  & (wcol[None, :] != rowpos[:, None]))
            mask = np.where(ok, maskvals[wcp][None, :], NEG).astype(np.float16)
            masks.append(mask.view(np.uint8).reshape(128, MB))
        o = D_IDM0[0]
        cw[:, o:o + IDB] = ident_bytes
        cw[:, o + IDB:o + IDB + MB] = masks[0]
        cw[:, D_M1[0]:D_M1[0] + MB] = masks[1]
        cw[:, D_M23[0]:D_M23[0] + MB] = masks[2]
        cw[:, D_M23[0] + MB:D_M23[0] + 2 * MB] = masks[3]
        in_maps.append({"cw": cw})
    return in_maps, order, sqs


def kernel(batch, labels, anchors=None, negatives=None, **_kwargs):
    batch = np.ascontiguousarray(np.asarray(batch), dtype=np.float32)
    labels_np = np.asarray(labels).astype(np.int64)
    negatives_np = np.asarray(negatives).astype(np.int64)

    in_maps, order, sqs = _prep_inputs(batch, labels_np)
    nc = get_nc()
    res = bass_utils.run_bass_kernel_spmd(nc, in_maps, core_ids=list(range(NCORES)))

    v = np.stack([np.asarray(r["out"], dtype=np.float64) for r in res.results])
    vsorted = v.transpose(0, 2, 1).reshape(B)     # [core, w, p] -> sorted pos
    d2ap_sorted = sqs + 512.0 - 2.0 * vsorted
    d2_ap = np.empty(B, dtype=np.float64)
    d2_ap[order] = d2ap_sorted
    d_ap = np.sqrt(np.maximum(d2_ap, 1e-12))

    diff = batch.astype(np.float64) - batch[negatives_np].astype(np.float64)
    d_an = np.sqrt(np.maximum(np.einsum("ij,ij->i", diff, diff), 1e-12))

    z = (d_ap - d_an) / (2.0 * TEMP)
    per = np.logaddexp(0.0, z)

    hist = np.bincount(labels_np, minlength=C)
    valid = (hist[labels_np] - 1) > 1
    count = float(valid.sum())
    loss = float((per * valid.astype(np.float64)).sum() / count)
    return np.array([loss], dtype=np.float32)
